# revision 5
# baseline (speedup 1.0000x reference)
"""Trainium2 Bass kernel for CrossAttention (B=32, N=M=1024, D=1024, DQK=128).

Computes, per batch b:
    Q = x @ Wq + bq            [N, DQK]
    K = ctx @ Wk + bk          [M, DQK]
    V = ctx @ Wv + bv          [M, D]
    S = Q @ K^T                [N, M]
    W = softmax(S, axis=-1)    [N, M]
    out = W @ V + x            [N, D]
Returns (out, W) as float32, matching the reference.

Sharding: data-parallel over batch across 8 NeuronCores (4 batches/core),
weights replicated. Each core runs an identical SPMD Bass/Tile program.

v2 design notes (vs the PE-transpose baseline at 487us):
- x and ctx are transposed on the HOST into [d, n]/[d, m] layout (and
  pre-rounded to tf32 so they can feed the PE as float32r directly), so the
  kernel needs ZERO PE transposes for its inputs.  The PE-transpose path was
  ~275ns per 128x128 tile x 768 tiles/core = ~210us of PE time.
- The attention-weight transpose (needed as the stationary operand of the
  W @ V matmul) goes through the DMA xbar transpose engine (SBUF->SBUF,
  bf16), not the PE.
- The output is computed TRANSPOSED (outT = (W@V)^T + xT = V^T@W^T + xT,
  accumulated per d-chunk) and un-transposed on the host, which lets the
  residual reuse the already-resident xT tile.
- Outputs (out, wts) are written as bf16 (host casts back to f32): halves
  output DMA; adds ~0.4% quantization, well within the 2e-2 gate
  (numpy-simulated end-to-end rel err: 3.7e-3).
- Q/K projections and scores stay f32r (tf32): a bf16 score path measures
  2.3e-2 rel err in simulation - over the gate.  V projection and W @ V run
  bf16 (fp8 also simulates over/near the gate, so not used).
"""

import numpy as np

B, N, M, D = 32, 1024, 1024, 1024
E = 128          # DQK
P = 128          # partitions
NCORES = 8
BPC = B // NCORES
KC = D // P      # contraction chunks
NC_ = N // P     # n chunks
MC = M // P      # m chunks
H = 512          # matmul moving free-dim (one PSUM bank of fp32)

_STATE = {}


def _tf32(a):
    """Round f32 to tf32 (10-bit mantissa) like the PE's f32r path."""
    ai = np.ascontiguousarray(a, dtype=np.float32).view(np.uint32)
    return ((ai + np.uint32(0x1000)) & np.uint32(0xFFFFE000)).view(np.float32)


def _build(nb):
    """Build the per-core Bass/Tile program for nb batches."""
    import concourse.bass as bass
    import concourse.tile as tile
    from concourse import bacc, mybir

    f32 = mybir.dt.float32
    f32r = mybir.dt.float32r
    bf16 = mybir.dt.bfloat16
    AX = mybir.AxisListType
    AF = mybir.ActivationFunctionType

    nc = bacc.Bacc(None, target_bir_lowering=False, debug=False)
    # host-transposed, tf32-prerounded inputs
    xT_d = nc.dram_tensor("xT", [nb, D, N], f32r, kind="ExternalInput")
    cT_d = nc.dram_tensor("ctxT", [nb, D, M], f32r, kind="ExternalInput")
    wq_d = nc.dram_tensor("Wq", [D, E], f32r, kind="ExternalInput")
    bq_d = nc.dram_tensor("bq", [E], f32, kind="ExternalInput")
    wk_d = nc.dram_tensor("Wk", [D, E], f32r, kind="ExternalInput")
    bk_d = nc.dram_tensor("bk", [E], f32, kind="ExternalInput")
    wv_d = nc.dram_tensor("Wv", [D, D], bf16, kind="ExternalInput")
    bv_d = nc.dram_tensor("bv", [D], f32, kind="ExternalInput")
    oT_d = nc.dram_tensor("outT", [nb, D, N], bf16, kind="ExternalOutput")
    wts_d = nc.dram_tensor("wts", [nb, N, M], bf16, kind="ExternalOutput")

    with tile.TileContext(nc) as tc:
        with (
            tc.tile_pool(name="const", bufs=1) as constp,
            tc.tile_pool(name="ctxp", bufs=1) as ctxp,
            tc.tile_pool(name="xp", bufs=2) as xp,
            tc.tile_pool(name="cbf", bufs=2) as cbfp,
            tc.tile_pool(name="vpool", bufs=1) as vpoolp,
            tc.tile_pool(name="qk", bufs=1) as qkp,
            tc.tile_pool(name="pwt", bufs=2) as pwtp,
            tc.tile_pool(name="soft", bufs=2) as softp,
            tc.tile_pool(name="outs", bufs=2) as outsp,
            tc.tile_pool(name="small", bufs=8) as smallp,
            tc.tile_pool(name="psum_mm", bufs=4, space="PSUM") as psmm,
        ):
            # ---- constants (loaded once), spread across DMA rings ----
            wq_sb = constp.tile([P, KC, E], f32r)
            nc.sync.dma_start(
                out=wq_sb, in_=wq_d[:, :].rearrange("(k p) e -> p k e", p=P)
            )
            wk_sb = constp.tile([P, KC, E], f32r)
            nc.sync.dma_start(
                out=wk_sb, in_=wk_d[:, :].rearrange("(k p) e -> p k e", p=P)
            )
            wv_bf = constp.tile([P, KC, D], bf16)
            nc.scalar.dma_start(
                out=wv_bf, in_=wv_d[:, :].rearrange("(k p) d -> p k d", p=P)
            )
            bq_sb = constp.tile([P, 1], f32)
            nc.scalar.dma_start(
                out=bq_sb, in_=bq_d[:].rearrange("(p one) -> p one", one=1)
            )
            bk_sb = constp.tile([P, 1], f32)
            nc.scalar.dma_start(
                out=bk_sb, in_=bk_d[:].rearrange("(p one) -> p one", one=1)
            )
            # bv broadcast to all partitions
            bv_sb = constp.tile([P, D], f32)
            bv_ap = bv_d[:]
            bv_bcast = bass.AP(
                tensor=bv_ap.tensor, offset=bv_ap.offset, ap=[[0, P]] + list(bv_ap.ap)
            )
            nc.gpsimd.dma_start(out=bv_sb, in_=bv_bcast)

            def emit_x_load(b):
                xT = xp.tile([P, KC, N], f32r, tag="xT")
                nc.sync.dma_start(
                    out=xT, in_=xT_d[b].rearrange("(k p) n -> p k n", p=P)
                )
                return xT

            xT_next = emit_x_load(0)

            for b in range(nb):
                xT = xT_next
                # ctxT single-buffered on the gpsimd (SWDGE) ring: its prefetch
                # is WAR-gated on batch b-1's last V-projection read, and that
                # wait must not head-of-line-block the other DMA rings.
                ctxT = ctxp.tile([P, KC, M], f32r, tag="ctxT")
                nc.gpsimd.dma_start(
                    out=ctxT, in_=cT_d[b].rearrange("(k p) m -> p k m", p=P)
                )

                # ---- K^T = (ctx @ Wk + bk)^T  -> [e, m] (f32r) ----
                k_ps = psmm.tile([P, M], f32, tag="mm")
                for h in range(2):
                    for k in range(KC):
                        nc.tensor.matmul(
                            k_ps[:, h * H : (h + 1) * H],
                            wk_sb[:, k, :],
                            ctxT[:, k, h * H : (h + 1) * H],
                            start=(k == 0),
                            stop=(k == KC - 1),
                        )
                kT = qkp.tile([P, M], f32r, tag="kT")
                nc.scalar.add(kT, k_ps, bk_sb)

                # ---- Q^T = (x @ Wq + bq)^T -> [e, n] (f32r) ----
                q_ps = psmm.tile([P, N], f32, tag="mm")
                for h in range(2):
                    for k in range(KC):
                        nc.tensor.matmul(
                            q_ps[:, h * H : (h + 1) * H],
                            wq_sb[:, k, :],
                            xT[:, k, h * H : (h + 1) * H],
                            start=(k == 0),
                            stop=(k == KC - 1),
                        )
                qT = qkp.tile([P, N], f32r, tag="qT")
                nc.scalar.add(qT, q_ps, bq_sb)

                # ---- scores/softmax interleaved with V-projection chunks ----
                # The softmax chain (ACT exp + normalize, DVE max/recip) takes
                # ~3us per n-chunk; interleaving one V-projection m-chunk (16
                # matmuls, ~3.4us) per softmax chunk keeps the PE busy and the
                # PSUM WAR rotation one chunk behind ACT.
                # pwT[p, i, j, f] = W[i*128+f, j*128+p]: the full transposed
                # weight matrix, filled per-i by the DMA xbar transpose.
                pwT = pwtp.tile([P, NC_, MC, P], bf16, tag="pwT")
                v_sb = vpoolp.tile([P, MC, D], bf16, tag="v")
                s_ps_list = [None] * NC_

                def emit_scores(i):
                    s_ps = psmm.tile([P, M], f32, tag="mm")
                    for h in range(2):
                        nc.tensor.matmul(
                            s_ps[:, h * H : (h + 1) * H],
                            qT[:, i * P : (i + 1) * P],
                            kT[:, h * H : (h + 1) * H],
                        )
                    return s_ps

                def emit_vproj(j):
                    cbf = cbfp.tile([P, KC, P], bf16, tag="cbf")
                    nc.vector.tensor_copy(cbf, ctxT[:, :, j * P : (j + 1) * P])
                    v_ps = psmm.tile([P, D], f32, tag="mm")
                    for h in range(2):
                        for k in range(KC):
                            nc.tensor.matmul(
                                v_ps[:, h * H : (h + 1) * H],
                                cbf[:, k, :],
                                wv_bf[:, k, h * H : (h + 1) * H],
                                start=(k == 0),
                                stop=(k == KC - 1),
                            )
                    nc.vector.tensor_add(v_sb[:, j, :], v_ps, bv_sb)

                s_ps_list[0] = emit_scores(0)
                for i in range(NC_):
                    if i + 1 < NC_:
                        s_ps_list[i + 1] = emit_scores(i + 1)
                    s_ps = s_ps_list[i]
                    s_ps_list[i] = None

                    negmax = smallp.tile([P, 1], f32, tag="negmax")
                    nc.vector.reduce_max(negmax, s_ps, axis=AX.X, negate=True)
                    p_sb = softp.tile([P, M], f32, tag="p")
                    sumex = smallp.tile([P, 1], f32, tag="sumex")
                    nc.scalar.activation(
                        p_sb, s_ps, AF.Exp, bias=negmax, scale=1.0, accum_out=sumex
                    )
                    rsum = smallp.tile([P, 1], f32, tag="rsum")
                    nc.vector.reciprocal(rsum, sumex)
                    # normalized weights in bf16: feeds the wts output, and the
                    # xbar transpose that builds the W@V stationary operand.
                    # Both DMAs dispatch from the sync ring - not ACT - so the
                    # softmax-chain engines stay pure compute.
                    pw_bf = softp.tile([P, M], bf16, tag="pwb")
                    nc.scalar.activation(
                        pw_bf, p_sb, AF.Identity, bias=0.0, scale=rsum
                    )
                    nc.sync.dma_start(
                        out=wts_d[b, i * P : (i + 1) * P, :], in_=pw_bf
                    )
                    nc.sync.dma_start_transpose(out=pwT[:, i, :, :], in_=pw_bf)
                    emit_vproj(i)

                # next batch's xT load goes into the sync FIFO here: after this
                # batch's xbar transposes, ahead of its outT writes, so it lands
                # during the attend phase (Q(b+1) needs it ~3.5us into b+1).
                if b + 1 < nb:
                    xT_next = emit_x_load(b + 1)

                # ---- outT[d, n] = V^T @ W^T + xT, per d-chunk ----
                for h2 in range(KC):
                    avT_ps = psmm.tile([P, N], f32, tag="mm")
                    for nh in range(2):
                        for j in range(MC):
                            nc.tensor.matmul(
                                avT_ps[:, nh * H : (nh + 1) * H],
                                v_sb[:, j, h2 * P : (h2 + 1) * P],
                                pwT[:, nh * 4 : (nh + 1) * 4, j, :],
                                start=(j == 0),
                                stop=(j == MC - 1),
                            )
                    oT_bf = outsp.tile([P, N], bf16, tag="o")
                    nc.vector.tensor_add(oT_bf, avT_ps, xT[:, h2, :])
                    nc.sync.dma_start(
                        out=oT_d[b, h2 * P : (h2 + 1) * P, :], in_=oT_bf
                    )

    return nc


def _get_program(nb):
    if nb not in _STATE:
        nc = _build(nb)
        nc.finalize()
        _STATE[nb] = nc
    return _STATE[nb]


def run(inputs, trace=False):
    """Run on 8 cores; returns (out, wts, BassKernelResults)."""
    import ml_dtypes
    from concourse import bass_utils

    nc = _get_program(BPC)
    x = np.ascontiguousarray(np.asarray(inputs["x"], dtype=np.float32))
    ctx = np.ascontiguousarray(np.asarray(inputs["context"], dtype=np.float32))
    # host-side: transpose to [b, d, n] and preround to tf32 for the f32r path
    xT = np.ascontiguousarray(_tf32(x).transpose(0, 2, 1))
    cT = np.ascontiguousarray(_tf32(ctx).transpose(0, 2, 1))
    shared = {
        "Wq": _tf32(inputs["Wq"]),
        "bq": np.ascontiguousarray(np.asarray(inputs["bq"], dtype=np.float32)),
        "Wk": _tf32(inputs["Wk"]),
        "bk": np.ascontiguousarray(np.asarray(inputs["bk"], dtype=np.float32)),
        "Wv": np.ascontiguousarray(
            np.asarray(inputs["Wv"], dtype=np.float32).astype(ml_dtypes.bfloat16)
        ),
        "bv": np.ascontiguousarray(np.asarray(inputs["bv"], dtype=np.float32)),
    }
    in_maps = []
    for c in range(NCORES):
        m = dict(shared)
        m["xT"] = xT[c * BPC : (c + 1) * BPC]
        m["ctxT"] = cT[c * BPC : (c + 1) * BPC]
        in_maps.append(m)

    kw = {}
    if trace:
        _install_ntff_hook()
        kw["trace"] = True
    res = bass_utils.run_bass_kernel_spmd(nc, in_maps, list(range(NCORES)), **kw)
    outT = np.concatenate(
        [np.asarray(res.results[c]["outT"]) for c in range(NCORES)], axis=0
    )
    wts = np.concatenate(
        [np.asarray(res.results[c]["wts"]) for c in range(NCORES)], axis=0
    )
    out = np.ascontiguousarray(outT.transpose(0, 2, 1)).astype(np.float32)
    wts = wts.astype(np.float32)
    return out, wts, res


def _install_ntff_hook():
    """The container's antenv stub lacks axon_hooks; provide it so
    run_bass_kernel_spmd(trace=True) can capture NTFF profiles."""
    import sys, types

    if "antenv.axon_hooks" in sys.modules:
        return
    import antenv
    from concourse import bass_utils

    bass_utils.upload_artifacts = lambda d: d  # no artifact store here
    try:
        from trn_agent_boot.trn_boot import _ntff_profile_via_ctypes

        hook = _ntff_profile_via_ctypes("/opt/axon/libaxon_pjrt.so")
    except Exception:
        hook = None
    mod = types.ModuleType("antenv.axon_hooks")
    mod.get_axon_ntff_profile_hook = lambda: hook
    mod.set_axon_ntff_profile_hook = lambda h: None
    sys.modules["antenv.axon_hooks"] = mod
    antenv.axon_hooks = mod


def kernel(**inputs):
    out, wts, _ = run(inputs, trace=False)
    return out, wts
